# revision 1
# baseline (speedup 1.0000x reference)
"""Trainium2 Bass kernel for nn_Attention_85057532330254.

Self-attention block (conv1x1 QKV + BatchNorm, relative-position bias,
softmax, gelu, out-projection + BatchNorm), batch-sharded across 8 cores.

Design (per core, 2 images = 2048 tokens):
 - x is PE-transposed on chip; Q^T/K^T/V^T computed directly in
   [channel, token] layout so BatchNorm stats are free-dim reductions and
   the BN affine is a per-partition scale/bias.
 - BN uses global batch stats -> two tiny AllReduces (qkv stats, z stats).
 - Softmax: exp(dots + bias) = exp(dots) * exp(bias); exp(bias) ("B") is
   precomputed on host (bf16), multiplied in on DVE at 2x bf16 rate.
 - Scores are built transposed (sT[j,i]) so attn@V needs no transposes;
   V_aug carries a ones-column producing softmax row-sums for free.
 - V's BN affine is folded into the gelu activation's per-partition
   scale/bias; attention output is built transposed (g^T) so the output
   projection needs no transpose either.
 - BN2 stats via ones-column matmul reductions; second AllReduce;
   final affine applied on DVE, result DMA'd out.
"""

import os

import numpy as np
import ml_dtypes

import concourse.bass as bass
import concourse.mybir as mybir
import concourse.tile as tile
from concourse import bacc
from concourse.bass import ts
from concourse.bass_utils import run_bass_kernel_spmd
from concourse.masks import make_identity

F32 = mybir.dt.float32
BF16 = mybir.dt.bfloat16
AF = mybir.ActivationFunctionType
ALU = mybir.AluOpType

FMAP = 32
HEADS = 8
DK = 32
DV = 64
EPS = 1e-5
N_TOK = FMAP * FMAP            # 1024 tokens per image
DIM = 256
INNER_K = HEADS * DK           # 256
INNER_V = HEADS * DV           # 512
SCALE = DK ** -0.5
NCORES = 8
IMGS = 2                        # images per core
TOKS = IMGS * N_TOK             # 2048
NTOT = float(16 * N_TOK)        # global batch size for BN stats

_cache = {}


def _build():
    from contextlib import ExitStack

    ndev = 1 if os.environ.get("KTIME") else NCORES
    nc = bacc.Bacc(
        "TRN2", target_bir_lowering=False, debug=False, num_devices=ndev
    )
    x_d = nc.dram_tensor("x", [TOKS, DIM], F32, kind="ExternalInput").ap()
    wqkv_d = nc.dram_tensor("wqkv", [DIM, 1024], F32, kind="ExternalInput").ap()
    gb_d = nc.dram_tensor("gb", [128, 16], F32, kind="ExternalInput").ap()
    bexp_d = nc.dram_tensor(
        "bexp", [HEADS, N_TOK, N_TOK], BF16, kind="ExternalInput"
    ).ap()
    wout_d = nc.dram_tensor("wout", [INNER_V, DIM], BF16, kind="ExternalInput").ap()
    vec2_d = nc.dram_tensor("vec2", [1, 3 * DIM], F32, kind="ExternalInput").ap()
    out_d = nc.dram_tensor("out", [TOKS, DIM], F32, kind="ExternalOutput").ap()

    with tile.TileContext(nc) as tc, ExitStack() as es:
        _kernel_body(tc, es, x_d, wqkv_d, gb_d, bexp_d, wout_d, vec2_d, out_d)
    nc.compile()
    return nc


def _kernel_body(tc, es, x_d, wqkv_d, gb_d, bexp_d, wout_d, vec2_d, out_d):
    nc = tc.nc
    RG = [list(range(NCORES))]

    const = es.enter_context(tc.tile_pool(name="const", bufs=1))
    ident = const.tile([128, 128], F32)
    make_identity(nc, ident)
    gb_sb = const.tile([128, 16], F32)
    nc.sync.dma_start(gb_sb[:], gb_d[:])
    vec2_sb = const.tile([1, 3 * DIM], F32)
    nc.sync.dma_start(vec2_sb[:], vec2_d[:])
    onescol = const.tile([128, 1], F32)
    nc.gpsimd.memset(onescol[:], 1.0)

    # persistent activations
    big = es.enter_context(tc.tile_pool(name="big", bufs=1))
    QKb = [big.tile([128, TOKS], BF16, tag=f"qkb{i}", name=f"qkb{i}") for i in range(4)]
    V_aug = big.tile([128, 16, HEADS, DV + 2], BF16, name="vaug")
    gT = [big.tile([64, TOKS], BF16, tag=f"gt{i}", name=f"gt{i}") for i in range(8)]
    z_sb = big.tile([128, 16 * DIM], F32, name="z_sb")
    stats_sb = const.tile([128, 16], F32)
    stats_all = const.tile([128, 16], F32)
    scale_t = const.tile([128, 8], F32)
    bias_t = const.tile([128, 8], F32)

    # ---------------- phase A/B: load x, transpose, project, stats --------
    xtp = tc.tile_pool(name="xtp", bufs=1)
    xtpool = xtp.__enter__()
    XT = [xtpool.tile([128, TOKS], F32, tag=f"xt{i}", name=f"xt{i}") for i in range(2)]
    with (
        tc.tile_pool(name="xnat", bufs=3) as xnat_pool,
        tc.tile_pool(name="trps", bufs=4, space="PSUM") as trps,
    ):
        for t in range(16):
            xn = xnat_pool.tile([128, DIM], F32)
            nc.sync.dma_start(xn[:], x_d[ts(t, 128), :])
            for fc in range(2):
                ps = trps.tile([128, 128], F32)
                nc.tensor.transpose(ps[:], xn[:, ts(fc, 128)], ident[:])
                nc.vector.tensor_copy(out=XT[fc][:, ts(t, 128)], in_=ps[:])

    wq_sb = [const.tile([128, 1024], F32, tag=f"wq{i}", name=f"wq{i}") for i in range(2)]
    for kc in range(2):
        nc.sync.dma_start(wq_sb[kc][:], wqkv_d[ts(kc, 128), :])
    wo_sb = [const.tile([64, DIM], BF16, tag=f"wo{i}", name=f"wo{i}") for i in range(8)]
    for dc in range(8):
        nc.sync.dma_start(wo_sb[dc][:], wout_d[ts(dc, 64), :])

    # projections chunk-by-chunk: c8 = q0 q1 k0 k1 v0 v1 v2 v3
    with (
        tc.tile_pool(name="qkraw", bufs=1) as qkraw_pool,
        tc.tile_pool(name="scratch", bufs=1) as scratch_pool,
    ):
        qkraw = []
        with tc.tile_pool(name="projps", bufs=2, space="PSUM") as projps:
          for c8 in range(8):
            ps = projps.tile([128, TOKS], F32, tag="proj")
            for ns in range(4):
                for kc in range(2):
                    nc.tensor.matmul(
                        ps[:, ts(ns, 512)],
                        lhsT=wq_sb[kc][:, ts(c8, 128)],
                        rhs=XT[kc][:, ts(ns, 512)],
                        start=(kc == 0),
                        stop=(kc == 1),
                    )
            scr = scratch_pool.tile([128, TOKS], BF16, tag="sq")
            nc.scalar.activation(
                out=scr[:], in_=ps[:], func=AF.Square,
                accum_out=stats_sb[:, 8 + c8:9 + c8],
            )
            nc.vector.tensor_reduce(
                out=stats_sb[:, c8:c8 + 1], in_=ps[:],
                axis=mybir.AxisListType.X, op=ALU.add,
            )
            if c8 < 4:
                raw = qkraw_pool.tile([128, TOKS], F32, tag=f"qk{c8}")
                nc.vector.tensor_copy(out=raw[:], in_=ps[:])
                qkraw.append(raw)

        # V natural (for attn@V lhsT): tiles [128tok, heads, 2+64]
        with tc.tile_pool(name="vps", bufs=2, space="PSUM") as vps:
            for t in range(16):
                ps = vps.tile([128, INNER_V], F32)
                for kc in range(2):
                    nc.tensor.matmul(
                        ps[:],
                        lhsT=XT[kc][:, ts(t, 128)],
                        rhs=wq_sb[kc][:, 512:1024],
                        start=(kc == 0),
                        stop=(kc == 1),
                    )
                nc.gpsimd.memset(V_aug[:, t], 1.0)
                nc.vector.tensor_copy(
                    out=V_aug[:, t, :, 1:65],
                    in_=ps.rearrange("p (h d) -> p h d", h=HEADS),
                )

        # ---- AllReduce 1: 2048 floats of (sum, sumsq) ----
        with tc.tile_pool(name="dram1", bufs=1, space="DRAM") as dram1:
            cin = dram1.tile([128, 16], F32)
            cout = dram1.tile([128, 16], F32)
            nc.sync.dma_start(cin[:], stats_sb[:])
            if os.environ.get("KTIME"):
                nc.sync.dma_start(cout[:], cin[:])
            else:
                nc.gpsimd.collective_compute(
                    "AllReduce", ALU.add, replica_groups=RG,
                    ins=[cin[:].opt()], outs=[cout[:].opt()],
                )
            nc.sync.dma_start(stats_all[:], cout[:])

        # ---- finalize BN1 affine: scale_t/bias_t [128, 8] ----
        mean = const.tile([128, 8], F32)
        ex2 = const.tile([128, 8], F32)
        veps = const.tile([128, 8], F32)
        sq0 = const.tile([128, 8], F32)
        tmp = const.tile([128, 8], F32)
        rstd = const.tile([128, 8], F32)
        nc.vector.tensor_scalar_mul(mean[:], stats_all[:, 0:8], 1.0 / NTOT)
        nc.vector.tensor_scalar_mul(ex2[:], stats_all[:, 8:16], 1.0 / NTOT)
        # veps = ex2 - mean^2 + eps
        nc.vector.scalar_tensor_tensor(
            out=tmp[:], in0=mean[:], scalar=-1.0, in1=mean[:],
            op0=ALU.mult, op1=ALU.mult,
        )
        nc.vector.tensor_add(veps[:], ex2[:], tmp[:])
        nc.vector.tensor_scalar_add(veps[:], veps[:], EPS)
        # sqrt + one Newton step: s = 0.5*(s0 + v/s0)
        nc.scalar.sqrt(sq0[:], veps[:])
        nc.vector.reciprocal(tmp[:], sq0[:])
        nc.vector.scalar_tensor_tensor(
            out=tmp[:], in0=veps[:], scalar=1.0, in1=tmp[:],
            op0=ALU.mult, op1=ALU.mult,
        )
        nc.vector.tensor_add(tmp[:], tmp[:], sq0[:])
        nc.vector.tensor_scalar_mul(tmp[:], tmp[:], 0.5)
        nc.vector.reciprocal(rstd[:], tmp[:])
        # scale = gamma * rstd ; bias = beta - mean * scale
        nc.vector.tensor_mul(scale_t[:], gb_sb[:, 0:8], rstd[:])
        nc.vector.scalar_tensor_tensor(
            out=tmp[:], in0=mean[:], scalar=-1.0, in1=scale_t[:],
            op0=ALU.mult, op1=ALU.mult,
        )
        nc.vector.tensor_add(bias_t[:], gb_sb[:, 8:16], tmp[:])
        # fold attention 1/sqrt(dk) into q
        nc.vector.tensor_scalar_mul(scale_t[:, 0:2], scale_t[:, 0:2], SCALE)
        nc.vector.tensor_scalar_mul(bias_t[:, 0:2], bias_t[:, 0:2], SCALE)

        # normalize Q/K -> bf16 (per-partition affine on ACT)
        for c8 in range(4):
            nc.scalar.activation(
                out=QKb[c8][:], in_=qkraw[c8][:], func=AF.Identity,
                bias=bias_t[:, c8:c8 + 1], scale=scale_t[:, c8:c8 + 1],
            )

        # repack per-head V scale/bias to partition base 0: col h = head h
        sv_pk = const.tile([64, 8], F32)
        bv_pk = const.tile([64, 8], F32)
        for h in range(HEADS):
            lo = 64 * (h % 2)
            c = 4 + h // 2
            nc.sync.dma_start(sv_pk[:, h:h + 1], scale_t[lo:lo + 64, c:c + 1])
            nc.sync.dma_start(bv_pk[:, h:h + 1], bias_t[lo:lo + 64, c:c + 1])

    xtp.__exit__(None, None, None)

    # ---------------- phase C: attention ----------------------------------
    with (
        tc.tile_pool(name="bpool", bufs=2) as bpool,
        tc.tile_pool(name="stpool", bufs=10) as stpool,
        tc.tile_pool(name="expool", bufs=2) as expool,
        tc.tile_pool(name="aps", bufs=2, space="PSUM") as aps,
        tc.tile_pool(name="small", bufs=2) as small,
    ):
        for h in range(HEADS):
            qk_t = h // 4
            hp = h % 4
            B_sb = bpool.tile([128, 8 * N_TOK], BF16, tag="B")
            nc.sync.dma_start(
                B_sb.rearrange("p (jc i) -> p jc i", jc=8),
                bexp_d[h].rearrange("(jc p) i -> p jc i", p=128),
            )
            sv_ap = sv_pk[:, h:h + 1]
            bv_ap = bv_pk[:, h:h + 1]
            sT = []
            for jc in range(8):
                st = stpool.tile([128, 2 * N_TOK], BF16, tag="sT")
                kpos = 32 * hp
                tp = (96, 0) if hp == 3 else None
                for img in range(IMGS):
                    dots = aps.tile([128, N_TOK], F32, tag="dots")
                    for ih in range(2):
                        nc.tensor.matmul(
                            dots[:, ts(ih, 512)],
                            lhsT=QKb[2 + qk_t][kpos:kpos + 32,
                                               img * N_TOK + jc * 128:
                                               img * N_TOK + jc * 128 + 128],
                            rhs=QKb[qk_t][kpos:kpos + 32,
                                          img * N_TOK + ih * 512:
                                          img * N_TOK + ih * 512 + 512],
                            start=True, stop=True,
                            tile_position=tp,
                        )
                    ex = expool.tile([128, N_TOK], BF16, tag="exp")
                    nc.scalar.activation(out=ex[:], in_=dots[:], func=AF.Exp)
                    nc.vector.tensor_mul(
                        st[:, ts(img, N_TOK)], ex[:], B_sb[:, ts(jc, N_TOK)],
                    )
                sT.append(st)
            for img in range(IMGS):
                # attn @ V_aug: rows 0..63 = dv, row 64 = rowsum (ones col)
                outp = aps.tile([128, N_TOK], F32, tag="outT", name="outp")
                rs_row = outp[64:65, :]
                for ih in range(2):
                    for jc in range(8):
                        nc.tensor.matmul(
                            outp[0:65, ts(ih, 512)],
                            lhsT=V_aug[:, img * 8 + jc, h, 1:66],
                            rhs=sT[jc][:, img * N_TOK + ih * 512:
                                       img * N_TOK + ih * 512 + 512],
                            start=(jc == 0), stop=(jc == 7),
                        )
                rsrow_sb = small.tile([1, N_TOK], F32, tag="rsrow")
                nc.vector.tensor_copy(out=rsrow_sb[:], in_=rs_row)
                rs = small.tile([8, 128], F32, tag="rs")
                nc.sync.dma_start(
                    rs[:], rsrow_sb.rearrange("o (p c) -> o p c", p=8)
                )
                rinv = small.tile([8, 128], F32, tag="rinv")
                nc.vector.reciprocal(rinv[:], rs[:])
                row = small.tile([1, N_TOK], F32, tag="row")
                nc.sync.dma_start(row[0:1, :], rinv[:])
                bc = small.tile([64, N_TOK], F32, tag="bc")
                nc.gpsimd.partition_broadcast(bc[:], row[0:1, :])
                xdiv = small.tile([64, N_TOK], BF16, tag="xdiv")
                nc.vector.tensor_mul(xdiv[:], outp[0:64, :], bc[:])
                nc.scalar.activation(
                    out=gT[h][:, ts(img, N_TOK)],
                    in_=xdiv[:],
                    func=AF.Gelu_apprx_tanh,
                    bias=bv_ap, scale=sv_ap,
                )

    # ---------------- phase D: out-projection + BN2 ------------------------
    with (
        tc.tile_pool(name="zps", bufs=2, space="PSUM") as zps,
        tc.tile_pool(name="sps", bufs=1, space="PSUM") as sps,
        tc.tile_pool(name="zmisc", bufs=2) as zmisc,
        tc.tile_pool(name="dram2", bufs=1, space="DRAM") as dram2,
        tc.tile_pool(name="fin", bufs=1) as fin,
    ):
        sums_ps = sps.tile([1, 2 * DIM], F32)
        for t in range(16):
            ps = zps.tile([128, DIM], F32, tag="z")
            for dc in range(8):
                nc.tensor.matmul(
                    ps[:],
                    lhsT=gT[dc][:, ts(t, 128)],
                    rhs=wo_sb[dc][:],
                    start=(dc == 0), stop=(dc == 7),
                )
            nc.vector.tensor_copy(out=z_sb[:, ts(t, DIM)], in_=ps[:])
            z2 = zmisc.tile([128, DIM], F32, tag="z2")
            nc.vector.tensor_mul(z2[:], z_sb[:, ts(t, DIM)], z_sb[:, ts(t, DIM)])
            nc.tensor.matmul(
                sums_ps[0:1, 0:DIM], lhsT=onescol[:], rhs=z_sb[:, ts(t, DIM)],
                start=(t == 0), stop=(t == 15), skip_group_check=True,
            )
            nc.tensor.matmul(
                sums_ps[0:1, DIM:2 * DIM], lhsT=onescol[:], rhs=z2[:],
                start=(t == 0), stop=(t == 15), skip_group_check=True,
            )
        st2 = fin.tile([1, 2 * DIM], F32)
        nc.vector.tensor_copy(out=st2[:], in_=sums_ps[:])
        cin = dram2.tile([1, 2 * DIM], F32)
        cout = dram2.tile([1, 2 * DIM], F32)
        nc.sync.dma_start(cin[:], st2[:])
        if os.environ.get("KTIME"):
            nc.sync.dma_start(cout[:], cin[:])
        else:
            nc.gpsimd.collective_compute(
                "AllReduce", ALU.add, replica_groups=RG,
                ins=[cin[:].opt()], outs=[cout[:].opt()],
            )
        st2a = fin.tile([1, 2 * DIM], F32)
        nc.sync.dma_start(st2a[:], cout[:])

        # finalize BN2 on [1, 256] rows.  z_true = z_raw + b_out
        mean = fin.tile([1, DIM], F32)
        ex2 = fin.tile([1, DIM], F32)
        veps = fin.tile([1, DIM], F32)
        sq0 = fin.tile([1, DIM], F32)
        tmp = fin.tile([1, DIM], F32)
        s2 = fin.tile([1, DIM], F32)
        b2f = fin.tile([1, DIM], F32)
        b_out_row = vec2_sb[0:1, 0:DIM]
        go_row = vec2_sb[0:1, DIM:2 * DIM]
        bo_row = vec2_sb[0:1, 2 * DIM:3 * DIM]
        nc.vector.tensor_scalar_mul(mean[:], st2a[0:1, 0:DIM], 1.0 / NTOT)
        nc.vector.tensor_scalar_mul(ex2[:], st2a[0:1, DIM:2 * DIM], 1.0 / NTOT)
        # ex2_true = ex2 + 2*mean*b_out + b_out^2 ; m_true = mean + b_out
        nc.vector.scalar_tensor_tensor(
            out=tmp[:], in0=mean[:], scalar=2.0, in1=b_out_row,
            op0=ALU.mult, op1=ALU.mult,
        )
        nc.vector.tensor_add(ex2[:], ex2[:], tmp[:])
        nc.vector.tensor_mul(tmp[:], b_out_row, b_out_row)
        nc.vector.tensor_add(ex2[:], ex2[:], tmp[:])
        m_true = fin.tile([1, DIM], F32)
        nc.vector.tensor_add(m_true[:], mean[:], b_out_row)
        nc.vector.scalar_tensor_tensor(
            out=tmp[:], in0=m_true[:], scalar=-1.0, in1=m_true[:],
            op0=ALU.mult, op1=ALU.mult,
        )
        nc.vector.tensor_add(veps[:], ex2[:], tmp[:])
        nc.vector.tensor_scalar_add(veps[:], veps[:], EPS)
        nc.scalar.sqrt(sq0[:], veps[:])
        nc.vector.reciprocal(tmp[:], sq0[:])
        nc.vector.scalar_tensor_tensor(
            out=tmp[:], in0=veps[:], scalar=1.0, in1=tmp[:],
            op0=ALU.mult, op1=ALU.mult,
        )
        nc.vector.tensor_add(tmp[:], tmp[:], sq0[:])
        nc.vector.tensor_scalar_mul(tmp[:], tmp[:], 0.5)
        nc.vector.reciprocal(tmp[:], tmp[:])        # rstd2
        nc.vector.tensor_mul(s2[:], go_row, tmp[:])
        # bias2_final = bo - mean_raw * s2
        nc.vector.scalar_tensor_tensor(
            out=tmp[:], in0=mean[:], scalar=-1.0, in1=s2[:],
            op0=ALU.mult, op1=ALU.mult,
        )
        nc.vector.tensor_add(b2f[:], bo_row, tmp[:])
        # widen scale/bias rows 16x (step-0 DMA), broadcast across partitions
        s2w = fin.tile([1, 16 * DIM], F32)
        b2w = fin.tile([1, 16 * DIM], F32)
        nc.sync.dma_start(
            s2w.rearrange("o (r c) -> o r c", r=16),
            s2.rearrange("o (u c) -> o u c", u=1).broadcast_to((1, 16, DIM)),
        )
        nc.sync.dma_start(
            b2w.rearrange("o (r c) -> o r c", r=16),
            b2f.rearrange("o (u c) -> o u c", u=1).broadcast_to((1, 16, DIM)),
        )
        bcs2 = fin.tile([128, 16 * DIM], F32)
        bcb2 = fin.tile([128, 16 * DIM], F32)
        nc.gpsimd.partition_broadcast(bcs2[:], s2w[:])
        nc.gpsimd.partition_broadcast(bcb2[:], b2w[:])
        zo = fin.tile([128, 16 * DIM], F32)
        nc.vector.tensor_mul(zo[:], z_sb[:], bcs2[:])
        nc.vector.tensor_add(zo[:], zo[:], bcb2[:])
        nc.sync.dma_start(
            out_d.rearrange("(t p) c -> p t c", p=128), zo.rearrange("p (t c) -> p t c", t=16)
        )


def _pos_idx():
    rng = np.arange(FMAP)
    pos = np.stack(np.meshgrid(rng, rng, indexing="ij"), -1).reshape(-1, 2)
    rel = np.abs(pos[:, None, :] - pos[None, :, :])
    return rel[..., 0] * FMAP + rel[..., 1]


def kernel(**inputs):
    f = np.float32
    x = np.asarray(inputs["x"], f)
    wq, wk, wv = (np.asarray(inputs[k], f) for k in ("wq", "wk", "wv"))
    pos_emb = np.asarray(inputs["pos_emb"], f)
    w_out = np.asarray(inputs["w_out"], f)

    bias = pos_emb[_pos_idx()]                       # [n, n, h]
    B = np.exp(bias.transpose(2, 0, 1) / SCALE).astype(ml_dtypes.bfloat16)

    wqkv = np.concatenate([wq, wk, wv], axis=1)      # [256, 1024]
    # gb: col 0-7 gammas, 8-15 betas, chunk order q0 q1 k0 k1 v0..v3
    gcat = np.concatenate(
        [np.asarray(inputs["gq"], f), np.asarray(inputs["gk"], f),
         np.asarray(inputs["gv"], f)]
    ).reshape(8, 128).T
    bcat = np.concatenate(
        [np.asarray(inputs["bq"], f), np.asarray(inputs["bk"], f),
         np.asarray(inputs["bv"], f)]
    ).reshape(8, 128).T
    gb = np.concatenate([gcat, bcat], axis=1).copy()  # [128, 16]
    vec2 = np.concatenate(
        [np.asarray(inputs["b_out"], f), np.asarray(inputs["go"], f),
         np.asarray(inputs["bo"], f)]
    )[None, :].copy()                                 # [1, 768]

    if "nc" not in _cache:
        _cache["nc"] = _build()
    nc = _cache["nc"]

    xs = x.reshape(16, N_TOK, DIM)
    in_maps = []
    for c in range(NCORES):
        in_maps.append({
            "x": np.ascontiguousarray(xs[2 * c:2 * c + 2].reshape(TOKS, DIM)),
            "wqkv": wqkv,
            "gb": gb,
            "bexp": B,
            "wout": w_out.astype(ml_dtypes.bfloat16),
            "vec2": vec2,
        })

    res = run_bass_kernel_spmd(
        nc, in_maps, core_ids=list(range(NCORES)),
        trace=bool(int(os.environ.get("KTRACE", "0"))),
    )
    _cache["res"] = res
    out = np.concatenate([r["out"] for r in res.results], axis=0)
    return out.reshape(16, FMAP, FMAP, DIM)


if __name__ == "__main__":
    if os.environ.get("BUILD_ONLY"):
        _build()
        print("BUILD OK")

